# revision 7
# baseline (speedup 1.0000x reference)
"""Trainium2 Bass kernel for BasicAttention (additive / Bahdanau attention).

Math (per batch b):
    h_proj = hidden @ Wh.T + bh          [S1, DM]
    f_proj = feats  @ Wf.T + bf          [S2, DM]
    scores[s,t] = sum_m v[m] * tanh(h_proj[s,m] + f_proj[t,m])   [S1, S2]
    weight = softmax(scores, axis=-1)
    context = weight @ feats             [S1, DF]
returns (context, weight)

Sharding: data-parallel over batch, one batch element per NeuronCore (B == 8).

Per-core engine plan:
  PE   : projections (fp32r), score reduction (tanh-tile stationary x v),
         transposes, context matmul
  DVE  : broadcast adds h_proj[s,:] + f_proj  (the tanh input)
  ACT  : tanh over S1*S2*DM elements (the bottleneck, ~1 elem/lane/cycle),
         softmax exp (same activation-table set as tanh)
All transposed layouts (hiddenT, featsT, WhT, WfT) are prepared on the host
for free so no on-chip transposes of the big operands are needed.
"""

import os
import sys

import numpy as np


def _ensure_concourse():
    try:
        import concourse.bass  # noqa: F401
        return
    except ImportError:
        pass
    for p in ("/opt/trn_rl_repo", "/root/.axon_site/_ro/trn_rl_repo"):
        if os.path.isdir(p) and p not in sys.path:
            sys.path.insert(0, p)
            try:
                import concourse.bass  # noqa: F401
                return
            except ImportError:
                continue
    raise ImportError("cannot locate concourse (bass) package")


_ensure_concourse()

import concourse.bacc as bacc  # noqa: E402
import concourse.tile as tile  # noqa: E402
from concourse import mybir  # noqa: E402
from concourse.bass_utils import run_bass_kernel_spmd  # noqa: E402
from concourse.masks import make_identity  # noqa: E402

# Problem shape (hardcoded per contest contract)
B, S1, S2 = 8, 64, 512
DH, DF, DM = 1024, 1024, 512

P = 128            # SBUF partitions
DK = DH // P       # 8 contraction k-tiles
MT = DM // P       # 4 m-tiles
TT = S2 // P       # 4 t-tiles
NCORES = 8

# Tunables
SGS = 8            # s-values per activation group (ACT free dim = SGS*512)
NSG = S1 // SGS

F32 = mybir.dt.float32
F32R = mybir.dt.float32r
BF16 = mybir.dt.bfloat16


def _r(ap):
    """View an fp32 access pattern as float32r (TF32-like full-rate matmul)."""
    return ap.bitcast(F32R)


def build_nc():
    nc = bacc.Bacc("TRN2", target_bir_lowering=False, debug=False,
                   num_devices=NCORES)

    # DRAM I/O (per-core shapes; host pre-transposes the big operands)
    hiddenT_d = nc.declare_dram_parameter("hiddenT", [DH, S1], F32R, isOutput=False)
    featsT_d = nc.declare_dram_parameter("featsT", [DF, S2], F32R, isOutput=False)
    feats_d = nc.declare_dram_parameter("feats", [S2, DF], F32R, isOutput=False)
    WhT_d = nc.declare_dram_parameter("WhT", [DH, DM], F32R, isOutput=False)
    WfT_d = nc.declare_dram_parameter("WfT", [DF, DM], F32R, isOutput=False)
    bhf_d = nc.declare_dram_parameter("bhfT", [P, MT], F32, isOutput=False)
    vT_d = nc.declare_dram_parameter("vT", [P, MT], F32, isOutput=False)
    ctx_d = nc.declare_dram_parameter("context", [S1, DF], F32, isOutput=True)
    wgt_d = nc.declare_dram_parameter("weight", [S1, S2], F32, isOutput=True)

    with tile.TileContext(nc) as tc:
        _build_body(nc, tc, hiddenT_d, featsT_d, feats_d, WhT_d, WfT_d,
                    bhf_d, vT_d, ctx_d, wgt_d)
    nc.compile()
    return nc


def _build_body(nc, tc, hiddenT_d, featsT_d, feats_d, WhT_d, WfT_d,
                bhf_d, vT_d, ctx_d, wgt_d):
    from contextlib import ExitStack
    ctx = ExitStack()
    with ctx:
        const = ctx.enter_context(tc.tile_pool(name="const", bufs=1))
        wpool = ctx.enter_context(tc.tile_pool(name="wpool", bufs=1))
        fpool = ctx.enter_context(tc.tile_pool(name="fpool", bufs=1))
        projp = ctx.enter_context(tc.tile_pool(name="projp", bufs=1))
        addp = ctx.enter_context(tc.tile_pool(name="addp", bufs=2))
        tanhp = ctx.enter_context(tc.tile_pool(name="tanhp", bufs=MT + 1))
        outp = ctx.enter_context(tc.tile_pool(name="outp", bufs=1))

        # ---- Phase 0: input DMAs + constants ----
        featsT_sb = []
        for k in range(DK):
            t = fpool.tile([P, S2], F32R, tag=f"featsT{k}")
            nc.sync.dma_start(out=t, in_=featsT_d[k * P:(k + 1) * P, :])
            featsT_sb.append(t)
        wft_sb = []
        for k in range(DK):
            t = wpool.tile([P, DM], F32R, tag=f"wft{k}")
            nc.sync.dma_start(out=t, in_=WfT_d[k * P:(k + 1) * P, :])
            wft_sb.append(t)
        wht_sb = []
        for k in range(DK):
            t = wpool.tile([P, DM], F32R, tag=f"wht{k}")
            nc.sync.dma_start(out=t, in_=WhT_d[k * P:(k + 1) * P, :])
            wht_sb.append(t)
        hiddenT_sb = []
        for k in range(DK):
            t = fpool.tile([P, S1], F32R, tag=f"hiddenT{k}")
            nc.sync.dma_start(out=t, in_=hiddenT_d[k * P:(k + 1) * P, :])
            hiddenT_sb.append(t)
        feats_sb = []
        for t4 in range(TT):
            t = fpool.tile([P, DF], F32R, tag=f"feats{t4}")
            nc.sync.dma_start(out=t, in_=feats_d[t4 * P:(t4 + 1) * P, :])
            feats_sb.append(t)

        bhf_sb = const.tile([P, MT], F32, tag="bhf")
        nc.sync.dma_start(out=bhf_sb, in_=bhf_d[:, :])
        vT_sb = const.tile([P, MT], F32, tag="vT")
        nc.sync.dma_start(out=vT_sb, in_=vT_d[:, :])
        v_bf = const.tile([P, MT], BF16, tag="v_bf")
        nc.vector.tensor_copy(out=v_bf, in_=vT_sb)

        ident = const.tile([P, P], F32, tag="ident")
        make_identity(nc, ident)

        # ---- Phase 1: projections (fp32r on PE) ----
        fproj_sb = []
        hproj_sb = []
        with tc.tile_pool(name="pp", bufs=2, space="PSUM") as pp, \
                tc.tile_pool(name="hp", bufs=2, space="PSUM") as hp:
            # f_projT[m] : [128(m), 512(t)] = WfT[:, m].T @ featsT
            for m in range(MT):
                fp_ps = pp.tile([P, S2], F32, tag="fp_ps")
                for k in range(DK):
                    nc.tensor.matmul(
                        fp_ps,
                        wft_sb[k][:, m * P:(m + 1) * P],
                        featsT_sb[k],
                        start=(k == 0), stop=(k == DK - 1),
                    )
                t = projp.tile([P, S2], F32, tag=f"fproj{m}")
                nc.vector.tensor_copy(out=t, in_=fp_ps)
                fproj_sb.append(t)

            # h_projT[m] : [128(m), 64(s)] = WhT[:, m].T @ hiddenT (+ bh + bf)
            for m in range(MT):
                hp_ps = hp.tile([P, S1], F32, tag="hp_ps")
                for k in range(DK):
                    nc.tensor.matmul(
                        hp_ps,
                        wht_sb[k][:, m * P:(m + 1) * P],
                        hiddenT_sb[k],
                        start=(k == 0), stop=(k == DK - 1),
                    )
                t = projp.tile([P, S1], F32, tag=f"hproj{m}")
                nc.vector.tensor_scalar_add(t, hp_ps, bhf_sb[:, m:m + 1])
                hproj_sb.append(t)

        with tc.tile_pool(name="scp", bufs=1, space="PSUM") as scp:
            # scores^T accumulators: 4 tiles [128(t), 64(s)]
            scT_ps = [scp.tile([P, S1], F32, tag=f"scT{t4}", name=f"scT{t4}")
                      for t4 in range(TT)]

            # ---- Phase 2: main loop — add, tanh, score matvecs ----
            # m-loop must be innermost for the PSUM accumulation (one
            # accumulation group at a time per psum tile), so keep all MT
            # tanh tiles of an s-group alive at once.
            for sg in range(NSG):
                tanh_ts = []
                for m in range(MT):
                    add_t = addp.tile([P, SGS * S2], F32, tag="add_t")
                    for j in range(SGS):
                        s = sg * SGS + j
                        nc.vector.tensor_scalar_add(
                            add_t[:, j * S2:(j + 1) * S2],
                            fproj_sb[m],
                            hproj_sb[m][:, s:s + 1],
                        )
                    tanh_t = tanhp.tile([P, SGS * S2], BF16, tag="tanh_t")
                    nc.scalar.activation(
                        out=tanh_t, in_=add_t,
                        func=mybir.ActivationFunctionType.Tanh,
                    )
                    tanh_ts.append(tanh_t)
                for j in range(SGS):
                    s = sg * SGS + j
                    for t4 in range(TT):
                        for m in range(MT):
                            nc.tensor.matmul(
                                scT_ps[t4][:, s:s + 1],
                                tanh_ts[m][:, j * S2 + t4 * P:
                                           j * S2 + (t4 + 1) * P],
                                v_bf[:, m:m + 1],
                                start=(m == 0), stop=(m == MT - 1),
                            )

            # copy scores^T out of PSUM (frees the scp banks)
            scT_sb = []
            for t4 in range(TT):
                t = outp.tile([P, S1], F32, tag=f"scTsb{t4}")
                nc.vector.tensor_copy(out=t, in_=scT_ps[t4])
                scT_sb.append(t)

        ep = ctx.enter_context(tc.tile_pool(name="ep", bufs=1, space="PSUM"))

        # ---- Phase 3: transpose scores^T -> scores, softmax ----
        scores_ps = ep.tile([S1, S2], F32, tag="scores_ps")
        for t4 in range(TT):
            nc.tensor.transpose(
                scores_ps[:, t4 * P:(t4 + 1) * P], scT_sb[t4], ident,
            )

        negmax = outp.tile([S1, 1], F32, tag="negmax")
        nc.vector.tensor_reduce(
            negmax, scores_ps, axis=mybir.AxisListType.X,
            op=mybir.AluOpType.max, negate=True,
        )
        exp_t = outp.tile([S1, S2], F32, tag="exp_t")
        sumexp = outp.tile([S1, 1], F32, tag="sumexp")
        nc.scalar.activation(
            out=exp_t, in_=scores_ps,
            func=mybir.ActivationFunctionType.Exp,
            bias=negmax, accum_out=sumexp,
        )
        rec = outp.tile([S1, 1], F32, tag="rec")
        nc.vector.reciprocal(rec, sumexp)
        weight_sb = outp.tile([S1, S2], F32, tag="weight_sb")
        nc.vector.tensor_scalar_mul(weight_sb, exp_t, rec)
        nc.gpsimd.dma_start(out=wgt_d[:, :], in_=weight_sb)

        # ---- Phase 4: context = weight @ feats ----
        wt_sb = []
        for t4 in range(TT):
            wt_ps = ep.tile([P, S1], F32, tag="wt_ps")
            nc.tensor.transpose(
                wt_ps, weight_sb[:, t4 * P:(t4 + 1) * P], ident[:S1, :S1],
            )
            t = outp.tile([P, S1], F32R, tag=f"wt{t4}")
            nc.vector.tensor_copy(out=t, in_=wt_ps)
            wt_sb.append(t)

        ctx_sb = outp.tile([S1, DF], F32, tag="ctx_sb")
        for h in range(2):
            ctx_ps = ep.tile([S1, 512], F32, tag="ctx_ps")
            for t4 in range(TT):
                nc.tensor.matmul(
                    ctx_ps,
                    wt_sb[t4],
                    feats_sb[t4][:, h * 512:(h + 1) * 512],
                    start=(t4 == 0), stop=(t4 == TT - 1),
                )
            nc.vector.tensor_copy(out=ctx_sb[:, h * 512:(h + 1) * 512], in_=ctx_ps)
        nc.gpsimd.dma_start(out=ctx_d[:, :], in_=ctx_sb)


_NC_CACHE = None


def _get_nc():
    global _NC_CACHE
    if _NC_CACHE is None:
        _NC_CACHE = build_nc()
    return _NC_CACHE


def _prep_in_maps(hidden_state, feats, Wh, bh, Wf, bf, v):
    hidden_state = np.asarray(hidden_state, dtype=np.float32)
    feats = np.asarray(feats, dtype=np.float32)
    Wh = np.asarray(Wh, dtype=np.float32)
    bh = np.asarray(bh, dtype=np.float32)
    Wf = np.asarray(Wf, dtype=np.float32)
    bf = np.asarray(bf, dtype=np.float32)
    v = np.asarray(v, dtype=np.float32)

    WhT = np.ascontiguousarray(Wh.T)                      # [DH, DM]
    WfT = np.ascontiguousarray(Wf.T)                      # [DF, DM]
    bhfT = np.ascontiguousarray((bh + bf).reshape(MT, P).T)  # [128, 4]
    vT = np.ascontiguousarray(v.reshape(MT, P).T)         # [128, 4]

    in_maps = []
    for b in range(NCORES):
        in_maps.append({
            "hiddenT": np.ascontiguousarray(hidden_state[b].T),
            "featsT": np.ascontiguousarray(feats[b].T),
            "feats": np.ascontiguousarray(feats[b]),
            "WhT": WhT,
            "WfT": WfT,
            "bhfT": bhfT,
            "vT": vT,
        })
    return in_maps


def kernel(hidden_state, feats, Wh, bh, Wf, bf, v, _run_kwargs=None):
    nc = _get_nc()
    in_maps = _prep_in_maps(hidden_state, feats, Wh, bh, Wf, bf, v)
    res = run_bass_kernel_spmd(nc, in_maps, list(range(NCORES)),
                               **(_run_kwargs or {}))
    context = np.stack([res.results[b]["context"] for b in range(NCORES)])
    weight = np.stack([res.results[b]["weight"] for b in range(NCORES)])
    kernel._last_results = res
    return context, weight


# revision 10
# speedup vs baseline: 1.0437x; 1.0437x over previous
"""Trainium2 Bass kernel for BasicAttention (additive / Bahdanau attention).

Math (per batch b):
    h_proj = hidden @ Wh.T + bh          [S1, DM]
    f_proj = feats  @ Wf.T + bf          [S2, DM]
    scores[s,t] = sum_m v[m] * tanh(h_proj[s,m] + f_proj[t,m])   [S1, S2]
    weight = softmax(scores, axis=-1)
    context = weight @ feats             [S1, DF]
returns (context, weight)

Sharding: data-parallel over batch, one batch element per NeuronCore (B == 8).

Per-core engine plan:
  PE   : projections (fp32r), score reduction (tanh-tile stationary x v),
         transposes, context matmul
  DVE  : broadcast adds h_proj[s,:] + f_proj  (the tanh input)
  ACT  : tanh over S1*S2*DM elements (the bottleneck, ~1 elem/lane/cycle),
         softmax exp (same activation-table set as tanh)
All transposed layouts (hiddenT, featsT, WhT, WfT) are prepared on the host
for free so no on-chip transposes of the big operands are needed.
"""

import os
import sys

import numpy as np


def _ensure_concourse():
    try:
        import concourse.bass  # noqa: F401
        return
    except ImportError:
        pass
    for p in ("/opt/trn_rl_repo", "/root/.axon_site/_ro/trn_rl_repo"):
        if os.path.isdir(p) and p not in sys.path:
            sys.path.insert(0, p)
            try:
                import concourse.bass  # noqa: F401
                return
            except ImportError:
                continue
    raise ImportError("cannot locate concourse (bass) package")


_ensure_concourse()

import concourse.bacc as bacc  # noqa: E402
import concourse.tile as tile  # noqa: E402
from concourse import mybir  # noqa: E402
from concourse.bass_utils import run_bass_kernel_spmd  # noqa: E402
from concourse.masks import make_identity  # noqa: E402

# Problem shape (hardcoded per contest contract)
B, S1, S2 = 8, 64, 512
DH, DF, DM = 1024, 1024, 512

P = 128            # SBUF partitions
DK = DH // P       # 8 contraction k-tiles
MT = DM // P       # 4 m-tiles
TT = S2 // P       # 4 t-tiles
NCORES = 8

# Tunables
SGS = 8            # s-values per activation group (ACT free dim = SGS*512)
NSG = S1 // SGS

F32 = mybir.dt.float32
F32R = mybir.dt.float32r
BF16 = mybir.dt.bfloat16


def _r(ap):
    """View an fp32 access pattern as float32r (TF32-like full-rate matmul)."""
    return ap.bitcast(F32R)


def build_nc():
    nc = bacc.Bacc("TRN2", target_bir_lowering=False, debug=False,
                   num_devices=NCORES)

    # DRAM I/O (per-core shapes; host pre-transposes the big operands)
    hiddenT_d = nc.declare_dram_parameter("hiddenT", [DH, S1], F32R, isOutput=False)
    featsT_d = nc.declare_dram_parameter("featsT", [DF, S2], F32R, isOutput=False)
    feats_d = nc.declare_dram_parameter("feats", [S2, DF], F32R, isOutput=False)
    WhT_d = nc.declare_dram_parameter("WhT", [DH, DM], F32R, isOutput=False)
    WfT_d = nc.declare_dram_parameter("WfT", [DF, DM], F32R, isOutput=False)
    bhf_d = nc.declare_dram_parameter("bhfT", [P, MT], F32, isOutput=False)
    vT_d = nc.declare_dram_parameter("vT", [P, MT], F32, isOutput=False)
    ctx_d = nc.declare_dram_parameter("context", [S1, DF], F32, isOutput=True)
    wgt_d = nc.declare_dram_parameter("weight", [S1, S2], F32, isOutput=True)

    with tile.TileContext(nc) as tc:
        _build_body(nc, tc, hiddenT_d, featsT_d, feats_d, WhT_d, WfT_d,
                    bhf_d, vT_d, ctx_d, wgt_d)
    nc.compile()
    return nc


def _build_body(nc, tc, hiddenT_d, featsT_d, feats_d, WhT_d, WfT_d,
                bhf_d, vT_d, ctx_d, wgt_d):
    from contextlib import ExitStack
    ctx = ExitStack()
    with ctx:
        const = ctx.enter_context(tc.tile_pool(name="const", bufs=1))
        wpool = ctx.enter_context(tc.tile_pool(name="wpool", bufs=1))
        fpool = ctx.enter_context(tc.tile_pool(name="fpool", bufs=1))
        projp = ctx.enter_context(tc.tile_pool(name="projp", bufs=1))
        addp = ctx.enter_context(tc.tile_pool(name="addp", bufs=3))
        tanhp = ctx.enter_context(tc.tile_pool(name="tanhp", bufs=MT + 1))
        outp = ctx.enter_context(tc.tile_pool(name="outp", bufs=1))

        # ---- Phase 0: input DMAs + constants ----
        # DMA order == queue completion order: the k-interleaved
        # featsT/WfT stream gates the f_proj matmuls, so it goes first;
        # the natural-layout feats (context input) is deferred into the
        # main loop.
        bhf_sb = const.tile([P, MT], F32, tag="bhf")
        nc.sync.dma_start(out=bhf_sb, in_=bhf_d[:, :])
        vT_sb = const.tile([P, MT], F32, tag="vT")
        nc.sync.dma_start(out=vT_sb, in_=vT_d[:, :])
        v_bf = const.tile([P, MT], BF16, tag="v_bf")
        nc.vector.tensor_copy(out=v_bf, in_=vT_sb)

        featsT_sb = []
        wft_sb = []
        for k in range(DK):
            t = fpool.tile([P, S2], F32R, tag=f"featsT{k}", name=f"featsT{k}")
            nc.sync.dma_start(out=t, in_=featsT_d[k * P:(k + 1) * P, :])
            featsT_sb.append(t)
            t = wpool.tile([P, DM], F32R, tag=f"wft{k}", name=f"wft{k}")
            nc.sync.dma_start(out=t, in_=WfT_d[k * P:(k + 1) * P, :])
            wft_sb.append(t)
        wht_sb = []
        hiddenT_sb = []
        for k in range(DK):
            t = wpool.tile([P, DM], F32R, tag=f"wht{k}", name=f"wht{k}")
            nc.sync.dma_start(out=t, in_=WhT_d[k * P:(k + 1) * P, :])
            wht_sb.append(t)
            t = fpool.tile([P, S1], F32R, tag=f"hiddenT{k}", name=f"hiddenT{k}")
            nc.sync.dma_start(out=t, in_=hiddenT_d[k * P:(k + 1) * P, :])
            hiddenT_sb.append(t)

        ident = const.tile([P, P], F32, tag="ident")
        make_identity(nc, ident)

        # ---- Phase 1: projections (fp32r on PE) ----
        fproj_sb = []
        hproj_sb = []
        with tc.tile_pool(name="pp", bufs=2, space="PSUM") as pp, \
                tc.tile_pool(name="hp", bufs=2, space="PSUM") as hp:
            # f_projT[m] : [128(m), 512(t)] = WfT[:, m].T @ featsT
            for m in range(MT):
                fp_ps = pp.tile([P, S2], F32, tag="fp_ps")
                for k in range(DK):
                    nc.tensor.matmul(
                        fp_ps,
                        wft_sb[k][:, m * P:(m + 1) * P],
                        featsT_sb[k],
                        start=(k == 0), stop=(k == DK - 1),
                    )
                t = projp.tile([P, S2], F32, tag=f"fproj{m}")
                nc.vector.tensor_copy(out=t, in_=fp_ps)
                fproj_sb.append(t)

            # h_projT[m] : [128(m), 64(s)] = WhT[:, m].T @ hiddenT (+ bh + bf)
            for m in range(MT):
                hp_ps = hp.tile([P, S1], F32, tag="hp_ps")
                for k in range(DK):
                    nc.tensor.matmul(
                        hp_ps,
                        wht_sb[k][:, m * P:(m + 1) * P],
                        hiddenT_sb[k],
                        start=(k == 0), stop=(k == DK - 1),
                    )
                t = projp.tile([P, S1], F32, tag=f"hproj{m}")
                nc.vector.tensor_scalar_add(t, hp_ps, bhf_sb[:, m:m + 1])
                hproj_sb.append(t)

        # natural-layout feats is only needed by the context matmul at the
        # very end — issue its DMAs after the prologue-critical ones
        feats_sb = []
        for t4 in range(TT):
            t = fpool.tile([P, DF], F32R, tag=f"feats{t4}", name=f"feats{t4}")
            nc.sync.dma_start(out=t, in_=feats_d[t4 * P:(t4 + 1) * P, :])
            feats_sb.append(t)

        with tc.tile_pool(name="scp", bufs=1, space="PSUM") as scp:
            # scores^T accumulators: 4 tiles [128(t), 64(s)]
            scT_ps = [scp.tile([P, S1], F32, tag=f"scT{t4}", name=f"scT{t4}")
                      for t4 in range(TT)]

            # ---- Phase 2: main loop — add, tanh, score matvecs ----
            # m-loop must be innermost for the PSUM accumulation (one
            # accumulation group at a time per psum tile), so keep all MT
            # tanh tiles of an s-group alive at once.
            for sg in range(NSG):
                tanh_ts = []
                for m in range(MT):
                    add_t = addp.tile([P, SGS * S2], F32, tag="add_t")
                    for j in range(SGS):
                        s = sg * SGS + j
                        nc.vector.tensor_scalar_add(
                            add_t[:, j * S2:(j + 1) * S2],
                            fproj_sb[m],
                            hproj_sb[m][:, s:s + 1],
                        )
                    tanh_t = tanhp.tile([P, SGS * S2], BF16, tag="tanh_t")
                    nc.scalar.activation(
                        out=tanh_t, in_=add_t,
                        func=mybir.ActivationFunctionType.Tanh,
                    )
                    tanh_ts.append(tanh_t)
                for j in range(SGS):
                    s = sg * SGS + j
                    for t4 in range(TT):
                        for m in range(MT):
                            nc.tensor.matmul(
                                scT_ps[t4][:, s:s + 1],
                                tanh_ts[m][:, j * S2 + t4 * P:
                                           j * S2 + (t4 + 1) * P],
                                v_bf[:, m:m + 1],
                                start=(m == 0), stop=(m == MT - 1),
                            )

            # copy scores^T out of PSUM (frees the scp banks)
            scT_sb = []
            for t4 in range(TT):
                t = outp.tile([P, S1], F32, tag=f"scTsb{t4}")
                nc.vector.tensor_copy(out=t, in_=scT_ps[t4])
                scT_sb.append(t)

        ep = ctx.enter_context(tc.tile_pool(name="ep", bufs=1, space="PSUM"))

        # ---- Phase 3: transpose scores^T -> scores, softmax ----
        scores_ps = ep.tile([S1, S2], F32, tag="scores_ps")
        for t4 in range(TT):
            nc.tensor.transpose(
                scores_ps[:, t4 * P:(t4 + 1) * P], scT_sb[t4], ident,
            )

        negmax = outp.tile([S1, 1], F32, tag="negmax")
        nc.vector.tensor_reduce(
            negmax, scores_ps, axis=mybir.AxisListType.X,
            op=mybir.AluOpType.max, negate=True,
        )
        exp_t = outp.tile([S1, S2], F32, tag="exp_t")
        sumexp = outp.tile([S1, 1], F32, tag="sumexp")
        nc.scalar.activation(
            out=exp_t, in_=scores_ps,
            func=mybir.ActivationFunctionType.Exp,
            bias=negmax, accum_out=sumexp,
        )
        rec = outp.tile([S1, 1], F32, tag="rec")
        nc.vector.reciprocal(rec, sumexp)
        weight_sb = outp.tile([S1, S2], F32, tag="weight_sb")
        nc.vector.tensor_scalar_mul(weight_sb, exp_t, rec)
        nc.gpsimd.dma_start(out=wgt_d[:, :], in_=weight_sb)

        # ---- Phase 4: context = weight @ feats ----
        wt_sb = []
        for t4 in range(TT):
            wt_ps = ep.tile([P, S1], F32, tag="wt_ps")
            nc.tensor.transpose(
                wt_ps, weight_sb[:, t4 * P:(t4 + 1) * P], ident[:S1, :S1],
            )
            t = outp.tile([P, S1], F32R, tag=f"wt{t4}")
            nc.vector.tensor_copy(out=t, in_=wt_ps)
            wt_sb.append(t)

        ctx_sb = outp.tile([S1, DF], F32, tag="ctx_sb")
        for h in range(2):
            ctx_ps = ep.tile([S1, 512], F32, tag="ctx_ps")
            for t4 in range(TT):
                nc.tensor.matmul(
                    ctx_ps,
                    wt_sb[t4],
                    feats_sb[t4][:, h * 512:(h + 1) * 512],
                    start=(t4 == 0), stop=(t4 == TT - 1),
                )
            nc.vector.tensor_copy(out=ctx_sb[:, h * 512:(h + 1) * 512], in_=ctx_ps)
        nc.gpsimd.dma_start(out=ctx_d[:, :], in_=ctx_sb)


_NC_CACHE = None


def _get_nc():
    global _NC_CACHE
    if _NC_CACHE is None:
        _NC_CACHE = build_nc()
    return _NC_CACHE


def _prep_in_maps(hidden_state, feats, Wh, bh, Wf, bf, v):
    hidden_state = np.asarray(hidden_state, dtype=np.float32)
    feats = np.asarray(feats, dtype=np.float32)
    Wh = np.asarray(Wh, dtype=np.float32)
    bh = np.asarray(bh, dtype=np.float32)
    Wf = np.asarray(Wf, dtype=np.float32)
    bf = np.asarray(bf, dtype=np.float32)
    v = np.asarray(v, dtype=np.float32)

    WhT = np.ascontiguousarray(Wh.T)                      # [DH, DM]
    WfT = np.ascontiguousarray(Wf.T)                      # [DF, DM]
    bhfT = np.ascontiguousarray((bh + bf).reshape(MT, P).T)  # [128, 4]
    vT = np.ascontiguousarray(v.reshape(MT, P).T)         # [128, 4]

    in_maps = []
    for b in range(NCORES):
        in_maps.append({
            "hiddenT": np.ascontiguousarray(hidden_state[b].T),
            "featsT": np.ascontiguousarray(feats[b].T),
            "feats": np.ascontiguousarray(feats[b]),
            "WhT": WhT,
            "WfT": WfT,
            "bhfT": bhfT,
            "vT": vT,
        })
    return in_maps


def kernel(hidden_state, feats, Wh, bh, Wf, bf, v, _run_kwargs=None):
    nc = _get_nc()
    in_maps = _prep_in_maps(hidden_state, feats, Wh, bh, Wf, bf, v)
    res = run_bass_kernel_spmd(nc, in_maps, list(range(NCORES)),
                               **(_run_kwargs or {}))
    context = np.stack([res.results[b]["context"] for b in range(NCORES)])
    weight = np.stack([res.results[b]["weight"] for b in range(NCORES)])
    kernel._last_results = res
    return context, weight


# revision 11
# speedup vs baseline: 1.0632x; 1.0187x over previous
"""Trainium2 Bass kernel for BasicAttention (additive / Bahdanau attention).

Math (per batch b):
    h_proj = hidden @ Wh.T + bh          [S1, DM]
    f_proj = feats  @ Wf.T + bf          [S2, DM]
    scores[s,t] = sum_m v[m] * tanh(h_proj[s,m] + f_proj[t,m])   [S1, S2]
    weight = softmax(scores, axis=-1)
    context = weight @ feats             [S1, DF]
returns (context, weight)

Sharding: data-parallel over batch, one batch element per NeuronCore (B == 8).

Per-core engine plan:
  PE   : projections (fp32r), score reduction (tanh-tile stationary x v),
         transposes, context matmul
  DVE  : broadcast adds h_proj[s,:] + f_proj  (the tanh input)
  ACT  : tanh over S1*S2*DM elements (the bottleneck, ~1 elem/lane/cycle),
         softmax exp (same activation-table set as tanh)
All transposed layouts (hiddenT, featsT, WhT, WfT) are prepared on the host
for free so no on-chip transposes of the big operands are needed.
"""

import os
import sys

import numpy as np
import ml_dtypes


def _ensure_concourse():
    try:
        import concourse.bass  # noqa: F401
        return
    except ImportError:
        pass
    for p in ("/opt/trn_rl_repo", "/root/.axon_site/_ro/trn_rl_repo"):
        if os.path.isdir(p) and p not in sys.path:
            sys.path.insert(0, p)
            try:
                import concourse.bass  # noqa: F401
                return
            except ImportError:
                continue
    raise ImportError("cannot locate concourse (bass) package")


_ensure_concourse()

import concourse.bacc as bacc  # noqa: E402
import concourse.tile as tile  # noqa: E402
from concourse import mybir  # noqa: E402
from concourse.bass_utils import run_bass_kernel_spmd  # noqa: E402
from concourse.masks import make_identity  # noqa: E402

# Problem shape (hardcoded per contest contract)
B, S1, S2 = 8, 64, 512
DH, DF, DM = 1024, 1024, 512

P = 128            # SBUF partitions
DK = DH // P       # 8 contraction k-tiles
MT = DM // P       # 4 m-tiles
TT = S2 // P       # 4 t-tiles
NCORES = 8

# Tunables
SGS = 8            # s-values per activation group (ACT free dim = SGS*512)
NSG = S1 // SGS

F32 = mybir.dt.float32
F32R = mybir.dt.float32r
BF16 = mybir.dt.bfloat16


def _r(ap):
    """View an fp32 access pattern as float32r (TF32-like full-rate matmul)."""
    return ap.bitcast(F32R)


def build_nc():
    nc = bacc.Bacc("TRN2", target_bir_lowering=False, debug=False,
                   num_devices=NCORES)

    # DRAM I/O (per-core shapes; host pre-transposes the big operands)
    hiddenT_d = nc.declare_dram_parameter("hiddenT", [DH, S1], BF16, isOutput=False)
    featsT_d = nc.declare_dram_parameter("featsT", [DF, S2], BF16, isOutput=False)
    feats_d = nc.declare_dram_parameter("feats", [S2, DF], BF16, isOutput=False)
    WhT_d = nc.declare_dram_parameter("WhT", [DH, DM], BF16, isOutput=False)
    WfT_d = nc.declare_dram_parameter("WfT", [DF, DM], BF16, isOutput=False)
    bhf_d = nc.declare_dram_parameter("bhfT", [P, MT], F32, isOutput=False)
    vT_d = nc.declare_dram_parameter("vT", [P, MT], F32, isOutput=False)
    ctx_d = nc.declare_dram_parameter("context", [S1, DF], F32, isOutput=True)
    wgt_d = nc.declare_dram_parameter("weight", [S1, S2], F32, isOutput=True)

    with tile.TileContext(nc) as tc:
        _build_body(nc, tc, hiddenT_d, featsT_d, feats_d, WhT_d, WfT_d,
                    bhf_d, vT_d, ctx_d, wgt_d)
    nc.compile()
    return nc


def _build_body(nc, tc, hiddenT_d, featsT_d, feats_d, WhT_d, WfT_d,
                bhf_d, vT_d, ctx_d, wgt_d):
    from contextlib import ExitStack
    ctx = ExitStack()
    with ctx:
        const = ctx.enter_context(tc.tile_pool(name="const", bufs=1))
        wpool = ctx.enter_context(tc.tile_pool(name="wpool", bufs=1))
        fpool = ctx.enter_context(tc.tile_pool(name="fpool", bufs=1))
        projp = ctx.enter_context(tc.tile_pool(name="projp", bufs=1))
        addp = ctx.enter_context(tc.tile_pool(name="addp", bufs=3))
        tanhp = ctx.enter_context(tc.tile_pool(name="tanhp", bufs=MT + 1))
        outp = ctx.enter_context(tc.tile_pool(name="outp", bufs=1))

        # ---- Phase 0: input DMAs + constants ----
        # DMA order == queue completion order: the k-interleaved
        # featsT/WfT stream gates the f_proj matmuls, so it goes first;
        # the natural-layout feats (context input) is deferred into the
        # main loop.
        bhf_sb = const.tile([P, MT], F32, tag="bhf")
        nc.sync.dma_start(out=bhf_sb, in_=bhf_d[:, :])
        vT_sb = const.tile([P, MT], F32, tag="vT")
        nc.sync.dma_start(out=vT_sb, in_=vT_d[:, :])
        v_bf = const.tile([P, MT], BF16, tag="v_bf")
        nc.vector.tensor_copy(out=v_bf, in_=vT_sb)

        featsT_sb = []
        wft_sb = []
        for k in range(DK):
            t = fpool.tile([P, S2], BF16, tag=f"featsT{k}", name=f"featsT{k}")
            nc.sync.dma_start(out=t, in_=featsT_d[k * P:(k + 1) * P, :])
            featsT_sb.append(t)
            t = wpool.tile([P, DM], BF16, tag=f"wft{k}", name=f"wft{k}")
            nc.sync.dma_start(out=t, in_=WfT_d[k * P:(k + 1) * P, :])
            wft_sb.append(t)
        wht_sb = []
        hiddenT_sb = []
        for k in range(DK):
            t = wpool.tile([P, DM], BF16, tag=f"wht{k}", name=f"wht{k}")
            nc.sync.dma_start(out=t, in_=WhT_d[k * P:(k + 1) * P, :])
            wht_sb.append(t)
            t = fpool.tile([P, S1], BF16, tag=f"hiddenT{k}", name=f"hiddenT{k}")
            nc.sync.dma_start(out=t, in_=hiddenT_d[k * P:(k + 1) * P, :])
            hiddenT_sb.append(t)

        ident = const.tile([P, P], F32, tag="ident")
        make_identity(nc, ident)

        # ---- Phase 1: projections (fp32r on PE) ----
        fproj_sb = []
        hproj_sb = []
        with tc.tile_pool(name="pp", bufs=2, space="PSUM") as pp, \
                tc.tile_pool(name="hp", bufs=2, space="PSUM") as hp:
            # f_projT[m] : [128(m), 512(t)] = WfT[:, m].T @ featsT
            for m in range(MT):
                fp_ps = pp.tile([P, S2], F32, tag="fp_ps")
                for k in range(DK):
                    nc.tensor.matmul(
                        fp_ps,
                        wft_sb[k][:, m * P:(m + 1) * P],
                        featsT_sb[k],
                        start=(k == 0), stop=(k == DK - 1),
                    )
                t = projp.tile([P, S2], F32, tag=f"fproj{m}")
                nc.vector.tensor_copy(out=t, in_=fp_ps)
                fproj_sb.append(t)

            # h_projT[m] : [128(m), 64(s)] = WhT[:, m].T @ hiddenT (+ bh + bf)
            for m in range(MT):
                hp_ps = hp.tile([P, S1], F32, tag="hp_ps")
                for k in range(DK):
                    nc.tensor.matmul(
                        hp_ps,
                        wht_sb[k][:, m * P:(m + 1) * P],
                        hiddenT_sb[k],
                        start=(k == 0), stop=(k == DK - 1),
                    )
                t = projp.tile([P, S1], F32, tag=f"hproj{m}")
                nc.vector.tensor_scalar_add(t, hp_ps, bhf_sb[:, m:m + 1])
                hproj_sb.append(t)

        # natural-layout feats is only needed by the context matmul at the
        # very end — issue its DMAs after the prologue-critical ones
        feats_sb = []
        for t4 in range(TT):
            t = fpool.tile([P, DF], BF16, tag=f"feats{t4}", name=f"feats{t4}")
            nc.sync.dma_start(out=t, in_=feats_d[t4 * P:(t4 + 1) * P, :])
            feats_sb.append(t)

        with tc.tile_pool(name="scp", bufs=1, space="PSUM") as scp:
            # scores^T accumulators: 4 tiles [128(t), 64(s)]
            scT_ps = [scp.tile([P, S1], F32, tag=f"scT{t4}", name=f"scT{t4}")
                      for t4 in range(TT)]

            # ---- Phase 2: main loop — add, tanh, score matvecs ----
            # m-loop must be innermost for the PSUM accumulation (one
            # accumulation group at a time per psum tile), so keep all MT
            # tanh tiles of an s-group alive at once.
            for sg in range(NSG):
                tanh_ts = []
                for m in range(MT):
                    add_t = addp.tile([P, SGS * S2], F32, tag="add_t")
                    for j in range(SGS):
                        s = sg * SGS + j
                        nc.vector.tensor_scalar_add(
                            add_t[:, j * S2:(j + 1) * S2],
                            fproj_sb[m],
                            hproj_sb[m][:, s:s + 1],
                        )
                    tanh_t = tanhp.tile([P, SGS * S2], BF16, tag="tanh_t")
                    nc.scalar.activation(
                        out=tanh_t, in_=add_t,
                        func=mybir.ActivationFunctionType.Tanh,
                    )
                    tanh_ts.append(tanh_t)
                for j in range(SGS):
                    s = sg * SGS + j
                    for t4 in range(TT):
                        for m in range(MT):
                            nc.tensor.matmul(
                                scT_ps[t4][:, s:s + 1],
                                tanh_ts[m][:, j * S2 + t4 * P:
                                           j * S2 + (t4 + 1) * P],
                                v_bf[:, m:m + 1],
                                start=(m == 0), stop=(m == MT - 1),
                            )

            # copy scores^T out of PSUM (frees the scp banks)
            scT_sb = []
            for t4 in range(TT):
                t = outp.tile([P, S1], F32, tag=f"scTsb{t4}")
                nc.vector.tensor_copy(out=t, in_=scT_ps[t4])
                scT_sb.append(t)

        ep = ctx.enter_context(tc.tile_pool(name="ep", bufs=1, space="PSUM"))

        # ---- Phase 3: transpose scores^T -> scores, softmax ----
        scores_ps = ep.tile([S1, S2], F32, tag="scores_ps")
        for t4 in range(TT):
            nc.tensor.transpose(
                scores_ps[:, t4 * P:(t4 + 1) * P], scT_sb[t4], ident,
            )

        negmax = outp.tile([S1, 1], F32, tag="negmax")
        nc.vector.tensor_reduce(
            negmax, scores_ps, axis=mybir.AxisListType.X,
            op=mybir.AluOpType.max, negate=True,
        )
        exp_t = outp.tile([S1, S2], F32, tag="exp_t")
        sumexp = outp.tile([S1, 1], F32, tag="sumexp")
        nc.scalar.activation(
            out=exp_t, in_=scores_ps,
            func=mybir.ActivationFunctionType.Exp,
            bias=negmax, accum_out=sumexp,
        )
        rec = outp.tile([S1, 1], F32, tag="rec")
        nc.vector.reciprocal(rec, sumexp)
        weight_sb = outp.tile([S1, S2], F32, tag="weight_sb")
        nc.vector.tensor_scalar_mul(weight_sb, exp_t, rec)
        nc.gpsimd.dma_start(out=wgt_d[:, :], in_=weight_sb)

        # ---- Phase 4: context = weight @ feats ----
        wt_sb = []
        for t4 in range(TT):
            wt_ps = ep.tile([P, S1], F32, tag="wt_ps")
            nc.tensor.transpose(
                wt_ps, weight_sb[:, t4 * P:(t4 + 1) * P], ident[:S1, :S1],
            )
            t = outp.tile([P, S1], BF16, tag=f"wt{t4}")
            nc.vector.tensor_copy(out=t, in_=wt_ps)
            wt_sb.append(t)

        ctx_sb = outp.tile([S1, DF], F32, tag="ctx_sb")
        for h in range(2):
            ctx_ps = ep.tile([S1, 512], F32, tag="ctx_ps")
            for t4 in range(TT):
                nc.tensor.matmul(
                    ctx_ps,
                    wt_sb[t4],
                    feats_sb[t4][:, h * 512:(h + 1) * 512],
                    start=(t4 == 0), stop=(t4 == TT - 1),
                )
            nc.vector.tensor_copy(out=ctx_sb[:, h * 512:(h + 1) * 512], in_=ctx_ps)
        nc.gpsimd.dma_start(out=ctx_d[:, :], in_=ctx_sb)


_NC_CACHE = None


def _get_nc():
    global _NC_CACHE
    if _NC_CACHE is None:
        _NC_CACHE = build_nc()
    return _NC_CACHE


def _prep_in_maps(hidden_state, feats, Wh, bh, Wf, bf, v):
    hidden_state = np.asarray(hidden_state, dtype=np.float32)
    feats = np.asarray(feats, dtype=np.float32)
    Wh = np.asarray(Wh, dtype=np.float32)
    bh = np.asarray(bh, dtype=np.float32)
    Wf = np.asarray(Wf, dtype=np.float32)
    bf = np.asarray(bf, dtype=np.float32)
    v = np.asarray(v, dtype=np.float32)

    WhT = np.ascontiguousarray(Wh.T)                      # [DH, DM]
    WfT = np.ascontiguousarray(Wf.T)                      # [DF, DM]
    WhT_bf = WhT.astype(ml_dtypes.bfloat16)
    WfT_bf = WfT.astype(ml_dtypes.bfloat16)
    bhfT = np.ascontiguousarray((bh + bf).reshape(MT, P).T)  # [128, 4]
    vT = np.ascontiguousarray(v.reshape(MT, P).T)         # [128, 4]

    in_maps = []
    for b in range(NCORES):
        in_maps.append({
            "hiddenT": np.ascontiguousarray(hidden_state[b].T).astype(ml_dtypes.bfloat16),
            "featsT": np.ascontiguousarray(feats[b].T).astype(ml_dtypes.bfloat16),
            "feats": np.ascontiguousarray(feats[b]).astype(ml_dtypes.bfloat16),
            "WhT": WhT_bf,
            "WfT": WfT_bf,
            "bhfT": bhfT,
            "vT": vT,
        })
    return in_maps


def kernel(hidden_state, feats, Wh, bh, Wf, bf, v, _run_kwargs=None):
    nc = _get_nc()
    in_maps = _prep_in_maps(hidden_state, feats, Wh, bh, Wf, bf, v)
    res = run_bass_kernel_spmd(nc, in_maps, list(range(NCORES)),
                               **(_run_kwargs or {}))
    context = np.stack([res.results[b]["context"] for b in range(NCORES)])
    weight = np.stack([res.results[b]["weight"] for b in range(NCORES)])
    kernel._last_results = res
    return context, weight


# revision 13
# speedup vs baseline: 1.1063x; 1.0405x over previous
"""Trainium2 Bass kernel for BasicAttention (additive / Bahdanau attention).

Math (per batch b):
    h_proj = hidden @ Wh.T + bh          [S1, DM]
    f_proj = feats  @ Wf.T + bf          [S2, DM]
    scores[s,t] = sum_m v[m] * tanh(h_proj[s,m] + f_proj[t,m])   [S1, S2]
    weight = softmax(scores, axis=-1)
    context = weight @ feats             [S1, DF]
returns (context, weight)

Sharding: data-parallel over batch, one batch element per NeuronCore (B == 8).

Per-core engine plan:
  PE   : projections (fp32r), score reduction (tanh-tile stationary x v),
         transposes, context matmul
  DVE  : broadcast adds h_proj[s,:] + f_proj  (the tanh input)
  ACT  : tanh over S1*S2*DM elements (the bottleneck, ~1 elem/lane/cycle),
         softmax exp (same activation-table set as tanh)
All transposed layouts (hiddenT, featsT, WhT, WfT) are prepared on the host
for free so no on-chip transposes of the big operands are needed.
"""

import os
import sys

import numpy as np
import ml_dtypes


def _ensure_concourse():
    try:
        import concourse.bass  # noqa: F401
        return
    except ImportError:
        pass
    for p in ("/opt/trn_rl_repo", "/root/.axon_site/_ro/trn_rl_repo"):
        if os.path.isdir(p) and p not in sys.path:
            sys.path.insert(0, p)
            try:
                import concourse.bass  # noqa: F401
                return
            except ImportError:
                continue
    raise ImportError("cannot locate concourse (bass) package")


_ensure_concourse()

import concourse.bacc as bacc  # noqa: E402
import concourse.tile as tile  # noqa: E402
from concourse import mybir  # noqa: E402
from concourse.bass_utils import run_bass_kernel_spmd  # noqa: E402
from concourse.masks import make_identity  # noqa: E402

# Problem shape (hardcoded per contest contract)
B, S1, S2 = 8, 64, 512
DH, DF, DM = 1024, 1024, 512

P = 128            # SBUF partitions
DK = DH // P       # 8 contraction k-tiles
MT = DM // P       # 4 m-tiles
TT = S2 // P       # 4 t-tiles
NCORES = 8

# Tunables
SGS = 8            # s-values per activation group (ACT free dim = SGS*512)
NSG = S1 // SGS

F32 = mybir.dt.float32
F32R = mybir.dt.float32r
BF16 = mybir.dt.bfloat16


def _r(ap):
    """View an fp32 access pattern as float32r (TF32-like full-rate matmul)."""
    return ap.bitcast(F32R)


def build_nc():
    nc = bacc.Bacc("TRN2", target_bir_lowering=False, debug=False,
                   num_devices=NCORES)

    # DRAM I/O (per-core shapes; host pre-transposes the big operands)
    hiddenT_d = nc.declare_dram_parameter("hiddenT", [DH, S1], BF16, isOutput=False)
    featsT_d = nc.declare_dram_parameter("featsT", [DF, S2], BF16, isOutput=False)
    feats_d = nc.declare_dram_parameter("feats", [S2, DF], BF16, isOutput=False)
    WhT_d = nc.declare_dram_parameter("WhT", [DH, DM], BF16, isOutput=False)
    WfT_d = nc.declare_dram_parameter("WfT", [DF, DM], BF16, isOutput=False)
    bhf_d = nc.declare_dram_parameter("bhfT", [P, MT], F32, isOutput=False)
    vT_d = nc.declare_dram_parameter("vT", [P, MT], F32, isOutput=False)
    ctx_d = nc.declare_dram_parameter("context", [S1, DF], F32, isOutput=True)
    wgt_d = nc.declare_dram_parameter("weight", [S1, S2], F32, isOutput=True)

    with tile.TileContext(nc) as tc:
        _build_body(nc, tc, hiddenT_d, featsT_d, feats_d, WhT_d, WfT_d,
                    bhf_d, vT_d, ctx_d, wgt_d)
    nc.compile()
    return nc


def _build_body(nc, tc, hiddenT_d, featsT_d, feats_d, WhT_d, WfT_d,
                bhf_d, vT_d, ctx_d, wgt_d):
    from contextlib import ExitStack
    ctx = ExitStack()
    with ctx:
        const = ctx.enter_context(tc.tile_pool(name="const", bufs=1))
        wpool = ctx.enter_context(tc.tile_pool(name="wpool", bufs=1))
        fpool = ctx.enter_context(tc.tile_pool(name="fpool", bufs=1))
        projp = ctx.enter_context(tc.tile_pool(name="projp", bufs=1))
        addp = ctx.enter_context(tc.tile_pool(name="addp", bufs=3))
        tanhp = ctx.enter_context(tc.tile_pool(name="tanhp", bufs=MT + 1))
        outp = ctx.enter_context(tc.tile_pool(name="outp", bufs=1))

        # ---- Phase 0: input DMAs + constants ----
        # DMA order == queue completion order: the k-interleaved
        # featsT/WfT stream gates the f_proj matmuls, so it goes first;
        # the natural-layout feats (context input) is deferred into the
        # main loop.
        bhf_sb = const.tile([P, MT], F32, tag="bhf")
        nc.sync.dma_start(out=bhf_sb, in_=bhf_d[:, :])
        vT_sb = const.tile([P, MT], F32, tag="vT")
        nc.sync.dma_start(out=vT_sb, in_=vT_d[:, :])
        v_bf = const.tile([P, MT], BF16, tag="v_bf")
        nc.vector.tensor_copy(out=v_bf, in_=vT_sb)

        # Batched input loads: one dma_start costs ~670ns of *serial* issue
        # time on the sync sequencer, so load each operand as one big
        # [128, k, free] tile and slice per k-tile on-chip.  featsT/WfT are
        # split in two halves so the f_proj matmuls can start early.
        HK = DK // 2
        featsT_t = []
        wft_t = []
        for h2 in range(2):
            t = fpool.tile([P, HK, S2], BF16, tag=f"featsT{h2}",
                           name=f"featsT{h2}")
            nc.sync.dma_start(
                out=t,
                in_=featsT_d.rearrange("(k p) t -> p k t", p=P)[
                    :, h2 * HK:(h2 + 1) * HK, :])
            featsT_t.append(t)
            t = wpool.tile([P, HK, DM], BF16, tag=f"wft{h2}", name=f"wft{h2}")
            nc.sync.dma_start(
                out=t,
                in_=WfT_d.rearrange("(k p) m -> p k m", p=P)[
                    :, h2 * HK:(h2 + 1) * HK, :])
            wft_t.append(t)
        featsT_sb = [featsT_t[k // HK][:, k % HK, :] for k in range(DK)]
        wft_sb = [wft_t[k // HK][:, k % HK, :] for k in range(DK)]

        wht_t = wpool.tile([P, DK, DM], BF16, tag="wht", name="wht")
        nc.sync.dma_start(out=wht_t, in_=WhT_d.rearrange("(k p) m -> p k m", p=P))
        wht_sb = [wht_t[:, k, :] for k in range(DK)]
        hiddenT_t = fpool.tile([P, DK, S1], BF16, tag="hiddenT", name="hiddenT")
        nc.sync.dma_start(out=hiddenT_t,
                          in_=hiddenT_d.rearrange("(k p) s -> p k s", p=P))
        hiddenT_sb = [hiddenT_t[:, k, :] for k in range(DK)]

        ident = const.tile([P, P], F32, tag="ident")
        make_identity(nc, ident)

        # ---- Phase 1: projections (fp32r on PE) ----
        fproj_sb = []
        hproj_sb = []
        with tc.tile_pool(name="pp", bufs=2, space="PSUM") as pp, \
                tc.tile_pool(name="hp", bufs=2, space="PSUM") as hp:
            # f_projT[m] : [128(m), 512(t)] = WfT[:, m].T @ featsT
            for m in range(MT):
                fp_ps = pp.tile([P, S2], F32, tag="fp_ps")
                for k in range(DK):
                    nc.tensor.matmul(
                        fp_ps,
                        wft_sb[k][:, m * P:(m + 1) * P],
                        featsT_sb[k],
                        start=(k == 0), stop=(k == DK - 1),
                    )
                t = projp.tile([P, S2], F32, tag=f"fproj{m}")
                nc.vector.tensor_copy(out=t, in_=fp_ps)
                fproj_sb.append(t)

            # h_projT[m] : [128(m), 64(s)] = WhT[:, m].T @ hiddenT (+ bh + bf)
            for m in range(MT):
                hp_ps = hp.tile([P, S1], F32, tag="hp_ps")
                for k in range(DK):
                    nc.tensor.matmul(
                        hp_ps,
                        wht_sb[k][:, m * P:(m + 1) * P],
                        hiddenT_sb[k],
                        start=(k == 0), stop=(k == DK - 1),
                    )
                t = projp.tile([P, S1], F32, tag=f"hproj{m}")
                nc.vector.tensor_scalar_add(t, hp_ps, bhf_sb[:, m:m + 1])
                hproj_sb.append(t)

        # natural-layout feats is only needed by the context matmul at the
        # very end — issue its DMA after the prologue-critical ones
        feats_t = fpool.tile([P, TT, DF], BF16, tag="featsN", name="featsN")
        nc.sync.dma_start(out=feats_t,
                          in_=feats_d.rearrange("(k p) d -> p k d", p=P))
        feats_sb = [feats_t[:, t4, :] for t4 in range(TT)]

        with tc.tile_pool(name="scp", bufs=1, space="PSUM") as scp:
            # scores^T accumulators: 4 tiles [128(t), 64(s)]
            scT_ps = [scp.tile([P, S1], F32, tag=f"scT{t4}", name=f"scT{t4}")
                      for t4 in range(TT)]

            # ---- Phase 2: main loop — add, tanh, score matvecs ----
            # m-loop must be innermost for the PSUM accumulation (one
            # accumulation group at a time per psum tile), so keep all MT
            # tanh tiles of an s-group alive at once.
            for sg in range(NSG):
                tanh_ts = []
                for m in range(MT):
                    add_t = addp.tile([P, SGS * S2], F32, tag="add_t")
                    for j in range(SGS):
                        s = sg * SGS + j
                        nc.vector.tensor_scalar_add(
                            add_t[:, j * S2:(j + 1) * S2],
                            fproj_sb[m],
                            hproj_sb[m][:, s:s + 1],
                        )
                    tanh_t = tanhp.tile([P, SGS * S2], BF16, tag="tanh_t")
                    nc.scalar.activation(
                        out=tanh_t, in_=add_t,
                        func=mybir.ActivationFunctionType.Tanh,
                    )
                    tanh_ts.append(tanh_t)
                for j in range(SGS):
                    s = sg * SGS + j
                    for t4 in range(TT):
                        for m in range(MT):
                            nc.tensor.matmul(
                                scT_ps[t4][:, s:s + 1],
                                tanh_ts[m][:, j * S2 + t4 * P:
                                           j * S2 + (t4 + 1) * P],
                                v_bf[:, m:m + 1],
                                start=(m == 0), stop=(m == MT - 1),
                            )

            # copy scores^T out of PSUM (frees the scp banks)
            scT_sb = []
            for t4 in range(TT):
                t = outp.tile([P, S1], F32, tag=f"scTsb{t4}")
                nc.vector.tensor_copy(out=t, in_=scT_ps[t4])
                scT_sb.append(t)

        ep = ctx.enter_context(tc.tile_pool(name="ep", bufs=1, space="PSUM"))

        # ---- Phase 3: transpose scores^T -> scores, softmax ----
        scores_ps = ep.tile([S1, S2], F32, tag="scores_ps")
        for t4 in range(TT):
            nc.tensor.transpose(
                scores_ps[:, t4 * P:(t4 + 1) * P], scT_sb[t4], ident,
            )

        negmax = outp.tile([S1, 1], F32, tag="negmax")
        nc.vector.tensor_reduce(
            negmax, scores_ps, axis=mybir.AxisListType.X,
            op=mybir.AluOpType.max, negate=True,
        )
        exp_t = outp.tile([S1, S2], F32, tag="exp_t")
        sumexp = outp.tile([S1, 1], F32, tag="sumexp")
        nc.scalar.activation(
            out=exp_t, in_=scores_ps,
            func=mybir.ActivationFunctionType.Exp,
            bias=negmax, accum_out=sumexp,
        )
        rec = outp.tile([S1, 1], F32, tag="rec")
        nc.vector.reciprocal(rec, sumexp)
        weight_sb = outp.tile([S1, S2], F32, tag="weight_sb")
        nc.vector.tensor_scalar_mul(weight_sb, exp_t, rec)
        nc.gpsimd.dma_start(out=wgt_d[:, :], in_=weight_sb)

        # ---- Phase 4: context = weight @ feats ----
        wt_sb = []
        for t4 in range(TT):
            wt_ps = ep.tile([P, S1], F32, tag="wt_ps")
            nc.tensor.transpose(
                wt_ps, weight_sb[:, t4 * P:(t4 + 1) * P], ident[:S1, :S1],
            )
            t = outp.tile([P, S1], BF16, tag=f"wt{t4}")
            nc.vector.tensor_copy(out=t, in_=wt_ps)
            wt_sb.append(t)

        ctx_sb = outp.tile([S1, DF], F32, tag="ctx_sb")
        for h in range(2):
            ctx_ps = ep.tile([S1, 512], F32, tag="ctx_ps")
            for t4 in range(TT):
                nc.tensor.matmul(
                    ctx_ps,
                    wt_sb[t4],
                    feats_sb[t4][:, h * 512:(h + 1) * 512],
                    start=(t4 == 0), stop=(t4 == TT - 1),
                )
            nc.vector.tensor_copy(out=ctx_sb[:, h * 512:(h + 1) * 512], in_=ctx_ps)
        nc.gpsimd.dma_start(out=ctx_d[:, :], in_=ctx_sb)


_NC_CACHE = None


def _get_nc():
    global _NC_CACHE
    if _NC_CACHE is None:
        _NC_CACHE = build_nc()
    return _NC_CACHE


def _prep_in_maps(hidden_state, feats, Wh, bh, Wf, bf, v):
    hidden_state = np.asarray(hidden_state, dtype=np.float32)
    feats = np.asarray(feats, dtype=np.float32)
    Wh = np.asarray(Wh, dtype=np.float32)
    bh = np.asarray(bh, dtype=np.float32)
    Wf = np.asarray(Wf, dtype=np.float32)
    bf = np.asarray(bf, dtype=np.float32)
    v = np.asarray(v, dtype=np.float32)

    WhT = np.ascontiguousarray(Wh.T)                      # [DH, DM]
    WfT = np.ascontiguousarray(Wf.T)                      # [DF, DM]
    WhT_bf = WhT.astype(ml_dtypes.bfloat16)
    WfT_bf = WfT.astype(ml_dtypes.bfloat16)
    bhfT = np.ascontiguousarray((bh + bf).reshape(MT, P).T)  # [128, 4]
    vT = np.ascontiguousarray(v.reshape(MT, P).T)         # [128, 4]

    in_maps = []
    for b in range(NCORES):
        in_maps.append({
            "hiddenT": np.ascontiguousarray(hidden_state[b].T).astype(ml_dtypes.bfloat16),
            "featsT": np.ascontiguousarray(feats[b].T).astype(ml_dtypes.bfloat16),
            "feats": np.ascontiguousarray(feats[b]).astype(ml_dtypes.bfloat16),
            "WhT": WhT_bf,
            "WfT": WfT_bf,
            "bhfT": bhfT,
            "vT": vT,
        })
    return in_maps


def kernel(hidden_state, feats, Wh, bh, Wf, bf, v, _run_kwargs=None):
    nc = _get_nc()
    in_maps = _prep_in_maps(hidden_state, feats, Wh, bh, Wf, bf, v)
    res = run_bass_kernel_spmd(nc, in_maps, list(range(NCORES)),
                               **(_run_kwargs or {}))
    context = np.stack([res.results[b]["context"] for b in range(NCORES)])
    weight = np.stack([res.results[b]["weight"] for b in range(NCORES)])
    kernel._last_results = res
    return context, weight


# revision 16
# speedup vs baseline: 1.1564x; 1.0453x over previous
"""Trainium2 Bass kernel for BasicAttention (additive / Bahdanau attention).

Math (per batch b):
    h_proj = hidden @ Wh.T + bh          [S1, DM]
    f_proj = feats  @ Wf.T + bf          [S2, DM]
    scores[s,t] = sum_m v[m] * tanh(h_proj[s,m] + f_proj[t,m])   [S1, S2]
    weight = softmax(scores, axis=-1)
    context = weight @ feats             [S1, DF]
returns (context, weight)

Sharding: data-parallel over batch, one batch element per NeuronCore (B == 8).

Per-core engine plan:
  PE   : projections (fp32r), score reduction (tanh-tile stationary x v),
         transposes, context matmul
  DVE  : broadcast adds h_proj[s,:] + f_proj  (the tanh input)
  ACT  : tanh over S1*S2*DM elements (the bottleneck, ~1 elem/lane/cycle),
         softmax exp (same activation-table set as tanh)
All transposed layouts (hiddenT, featsT, WhT, WfT) are prepared on the host
for free so no on-chip transposes of the big operands are needed.
"""

import os
import sys

import numpy as np
import ml_dtypes


def _ensure_concourse():
    try:
        import concourse.bass  # noqa: F401
        return
    except ImportError:
        pass
    for p in ("/opt/trn_rl_repo", "/root/.axon_site/_ro/trn_rl_repo"):
        if os.path.isdir(p) and p not in sys.path:
            sys.path.insert(0, p)
            try:
                import concourse.bass  # noqa: F401
                return
            except ImportError:
                continue
    raise ImportError("cannot locate concourse (bass) package")


_ensure_concourse()

import concourse.bacc as bacc  # noqa: E402
import concourse.tile as tile  # noqa: E402
from concourse import mybir  # noqa: E402
from concourse.bass_utils import run_bass_kernel_spmd  # noqa: E402
from concourse.masks import make_identity  # noqa: E402

# Problem shape (hardcoded per contest contract)
B, S1, S2 = 8, 64, 512
DH, DF, DM = 1024, 1024, 512

P = 128            # SBUF partitions
DK = DH // P       # 8 contraction k-tiles
MT = DM // P       # 4 m-tiles
TT = S2 // P       # 4 t-tiles
NCORES = 8

# Tunables
SGS = 8            # s-values per activation group (ACT free dim = SGS*512)
NSG = S1 // SGS

F32 = mybir.dt.float32
F32R = mybir.dt.float32r
BF16 = mybir.dt.bfloat16


def _r(ap):
    """View an fp32 access pattern as float32r (TF32-like full-rate matmul)."""
    return ap.bitcast(F32R)


def build_nc():
    nc = bacc.Bacc("TRN2", target_bir_lowering=False, debug=False,
                   num_devices=NCORES)

    # DRAM I/O (per-core shapes; host pre-transposes the big operands)
    hiddenT_d = nc.declare_dram_parameter("hiddenT", [DH, S1], BF16, isOutput=False)
    featsT_d = nc.declare_dram_parameter("featsT", [DF, S2], BF16, isOutput=False)
    feats_d = nc.declare_dram_parameter("feats", [S2, DF], BF16, isOutput=False)
    WhT_d = nc.declare_dram_parameter("WhT", [DH, DM], BF16, isOutput=False)
    WfT_d = nc.declare_dram_parameter("WfT", [DF, DM], BF16, isOutput=False)
    bhf_d = nc.declare_dram_parameter("bhfT", [P, MT], F32, isOutput=False)
    vT_d = nc.declare_dram_parameter("vT", [P, MT], F32, isOutput=False)
    ctx_d = nc.declare_dram_parameter("context", [S1, DF], F32, isOutput=True)
    wgt_d = nc.declare_dram_parameter("weight", [S1, S2], F32, isOutput=True)

    with tile.TileContext(nc) as tc:
        _build_body(nc, tc, hiddenT_d, featsT_d, feats_d, WhT_d, WfT_d,
                    bhf_d, vT_d, ctx_d, wgt_d)
    nc.compile()
    return nc


def _build_body(nc, tc, hiddenT_d, featsT_d, feats_d, WhT_d, WfT_d,
                bhf_d, vT_d, ctx_d, wgt_d):
    from contextlib import ExitStack
    ctx = ExitStack()
    with ctx:
        const = ctx.enter_context(tc.tile_pool(name="const", bufs=1))
        wpool = ctx.enter_context(tc.tile_pool(name="wpool", bufs=1))
        fpool = ctx.enter_context(tc.tile_pool(name="fpool", bufs=1))
        projp = ctx.enter_context(tc.tile_pool(name="projp", bufs=1))
        addp = ctx.enter_context(tc.tile_pool(name="addp", bufs=3))
        tanhp = ctx.enter_context(tc.tile_pool(name="tanhp", bufs=MT + 2))
        outp = ctx.enter_context(tc.tile_pool(name="outp", bufs=1))

        # ---- Phase 0: input DMAs + constants ----
        # DMA order == queue completion order: the k-interleaved
        # featsT/WfT stream gates the f_proj matmuls, so it goes first;
        # the natural-layout feats (context input) is deferred into the
        # main loop.
        bhf_sb = const.tile([P, MT], F32, tag="bhf")
        nc.sync.dma_start(out=bhf_sb, in_=bhf_d[:, :])
        vT_sb = const.tile([P, MT], F32, tag="vT")
        nc.sync.dma_start(out=vT_sb, in_=vT_d[:, :])
        v_bf = const.tile([P, MT], BF16, tag="v_bf")
        nc.vector.tensor_copy(out=v_bf, in_=vT_sb)

        # Batched input loads: one dma_start costs ~670ns of *serial* issue
        # time on the sync sequencer, so load each operand as one big
        # [128, k, free] tile and slice per k-tile on-chip.  featsT/WfT are
        # split in two halves so the f_proj matmuls can start early.
        HK = 2  # k-tiles per DMA chunk
        NCH = DK // HK
        featsT_t = []
        wft_t = []
        for h2 in range(NCH):
            t = fpool.tile([P, HK, S2], BF16, tag=f"featsT{h2}",
                           name=f"featsT{h2}")
            nc.sync.dma_start(
                out=t,
                in_=featsT_d.rearrange("(k p) t -> p k t", p=P)[
                    :, h2 * HK:(h2 + 1) * HK, :])
            featsT_t.append(t)
            t = wpool.tile([P, HK, DM], BF16, tag=f"wft{h2}", name=f"wft{h2}")
            nc.sync.dma_start(
                out=t,
                in_=WfT_d.rearrange("(k p) m -> p k m", p=P)[
                    :, h2 * HK:(h2 + 1) * HK, :])
            wft_t.append(t)
        featsT_sb = [featsT_t[k // HK][:, k % HK, :] for k in range(DK)]
        wft_sb = [wft_t[k // HK][:, k % HK, :] for k in range(DK)]

        wht_t = wpool.tile([P, DK, DM], BF16, tag="wht", name="wht")
        nc.sync.dma_start(out=wht_t, in_=WhT_d.rearrange("(k p) m -> p k m", p=P))
        wht_sb = [wht_t[:, k, :] for k in range(DK)]
        hiddenT_t = fpool.tile([P, DK, S1], BF16, tag="hiddenT", name="hiddenT")
        nc.sync.dma_start(out=hiddenT_t,
                          in_=hiddenT_d.rearrange("(k p) s -> p k s", p=P))
        hiddenT_sb = [hiddenT_t[:, k, :] for k in range(DK)]

        ident = const.tile([P, P], F32, tag="ident")
        make_identity(nc, ident)

        # ---- Phase 1: projections (bf16 on PE), m-interleaved so the
        # main loop can start as soon as fproj[0]/hproj[0] are done ----
        fproj_sb = []
        hproj_sb = []
        with tc.tile_pool(name="pp", bufs=2, space="PSUM") as pp, \
                tc.tile_pool(name="hp", bufs=2, space="PSUM") as hp:
            for m in range(MT):
                # f_projT[m] : [128(m), 512(t)] = WfT[:, m].T @ featsT
                fp_ps = pp.tile([P, S2], F32, tag="fp_ps")
                for k in range(DK):
                    nc.tensor.matmul(
                        fp_ps,
                        wft_sb[k][:, m * P:(m + 1) * P],
                        featsT_sb[k],
                        start=(k == 0), stop=(k == DK - 1),
                    )
                t = projp.tile([P, S2], F32, tag=f"fproj{m}")
                nc.scalar.copy(out=t, in_=fp_ps)
                fproj_sb.append(t)

                # h_projT[m] : [128(m), 64(s)] = WhT[:,m].T @ hiddenT (+bh+bf)
                hp_ps = hp.tile([P, S1], F32, tag="hp_ps")
                for k in range(DK):
                    nc.tensor.matmul(
                        hp_ps,
                        wht_sb[k][:, m * P:(m + 1) * P],
                        hiddenT_sb[k],
                        start=(k == 0), stop=(k == DK - 1),
                    )
                t = projp.tile([P, S1], F32, tag=f"hproj{m}")
                nc.vector.tensor_scalar_add(t, hp_ps, bhf_sb[:, m:m + 1])
                hproj_sb.append(t)

        # natural-layout feats is only needed by the context matmul at the
        # very end — issue its DMA after the prologue-critical ones
        feats_t = fpool.tile([P, TT, DF], BF16, tag="featsN", name="featsN")
        nc.sync.dma_start(out=feats_t,
                          in_=feats_d.rearrange("(k p) d -> p k d", p=P))
        feats_sb = [feats_t[:, t4, :] for t4 in range(TT)]

        with tc.tile_pool(name="scp", bufs=1, space="PSUM") as scp:
            # scores^T accumulators, split in two halves over m (8 banks):
            # the m0/m1 half only depends on the first two tanh tiles of an
            # s-group, so its matvecs overlap the m2/m3 tanh work.
            scT_a = [scp.tile([P, S1], F32, tag=f"scTa{t4}", name=f"scTa{t4}")
                     for t4 in range(TT)]
            scT_b = [scp.tile([P, S1], F32, tag=f"scTb{t4}", name=f"scTb{t4}")
                     for t4 in range(TT)]

            # ---- Phase 2: main loop — add, tanh, score matvecs ----
            # m-loop must be innermost for the PSUM accumulation (one
            # accumulation group at a time per psum tile), so keep all MT
            # tanh tiles of an s-group alive at once.
            for sg in range(NSG):
                tanh_ts = []
                for m in range(MT):
                    add_t = addp.tile([P, SGS * S2], F32, tag="add_t")
                    for j in range(SGS):
                        s = sg * SGS + j
                        nc.vector.tensor_scalar_add(
                            add_t[:, j * S2:(j + 1) * S2],
                            fproj_sb[m],
                            hproj_sb[m][:, s:s + 1],
                        )
                    tanh_t = tanhp.tile([P, SGS * S2], BF16, tag="tanh_t")
                    nc.scalar.activation(
                        out=tanh_t, in_=add_t,
                        func=mybir.ActivationFunctionType.Tanh,
                    )
                    tanh_ts.append(tanh_t)
                for j in range(SGS):
                    s = sg * SGS + j
                    for t4 in range(TT):
                        for half, ps in ((0, scT_a), (1, scT_b)):
                            for mi in range(2):
                                m = half * 2 + mi
                                nc.tensor.matmul(
                                    ps[t4][:, s:s + 1],
                                    tanh_ts[m][:, j * S2 + t4 * P:
                                               j * S2 + (t4 + 1) * P],
                                    v_bf[:, m:m + 1],
                                    start=(mi == 0), stop=(mi == 1),
                                )

            # merge the two halves out of PSUM (frees the scp banks)
            scT_sb = []
            for t4 in range(TT):
                t = outp.tile([P, S1], F32, tag=f"scTsb{t4}")
                nc.vector.tensor_copy(out=t, in_=scT_a[t4])
                nc.vector.tensor_add(t, t, scT_b[t4])
                scT_sb.append(t)

        ep = ctx.enter_context(tc.tile_pool(name="ep", bufs=1, space="PSUM"))

        # ---- Phase 3: transpose scores^T -> scores, softmax ----
        scores_ps = ep.tile([S1, S2], F32, tag="scores_ps")
        for t4 in range(TT):
            nc.tensor.transpose(
                scores_ps[:, t4 * P:(t4 + 1) * P], scT_sb[t4], ident,
            )

        negmax = outp.tile([S1, 1], F32, tag="negmax")
        nc.vector.tensor_reduce(
            negmax, scores_ps, axis=mybir.AxisListType.X,
            op=mybir.AluOpType.max, negate=True,
        )
        exp_t = outp.tile([S1, S2], F32, tag="exp_t")
        sumexp = outp.tile([S1, 1], F32, tag="sumexp")
        nc.scalar.activation(
            out=exp_t, in_=scores_ps,
            func=mybir.ActivationFunctionType.Exp,
            bias=negmax, accum_out=sumexp,
        )
        rec = outp.tile([S1, 1], F32, tag="rec")
        nc.vector.reciprocal(rec, sumexp)
        weight_sb = outp.tile([S1, S2], F32, tag="weight_sb")
        nc.vector.tensor_scalar_mul(weight_sb, exp_t, rec)
        nc.gpsimd.dma_start(out=wgt_d[:, :], in_=weight_sb)

        # ---- Phase 4: context = weight @ feats ----
        wt_sb = []
        for t4 in range(TT):
            wt_ps = ep.tile([P, S1], F32, tag="wt_ps")
            nc.tensor.transpose(
                wt_ps, weight_sb[:, t4 * P:(t4 + 1) * P], ident[:S1, :S1],
            )
            t = outp.tile([P, S1], BF16, tag=f"wt{t4}")
            nc.vector.tensor_copy(out=t, in_=wt_ps)
            wt_sb.append(t)

        ctx_sb = outp.tile([S1, DF], F32, tag="ctx_sb")
        for h in range(2):
            ctx_ps = ep.tile([S1, 512], F32, tag="ctx_ps")
            for t4 in range(TT):
                nc.tensor.matmul(
                    ctx_ps,
                    wt_sb[t4],
                    feats_sb[t4][:, h * 512:(h + 1) * 512],
                    start=(t4 == 0), stop=(t4 == TT - 1),
                )
            nc.vector.tensor_copy(out=ctx_sb[:, h * 512:(h + 1) * 512], in_=ctx_ps)
        nc.gpsimd.dma_start(out=ctx_d[:, :], in_=ctx_sb)


_NC_CACHE = None


def _get_nc():
    global _NC_CACHE
    if _NC_CACHE is None:
        _NC_CACHE = build_nc()
    return _NC_CACHE


def _prep_in_maps(hidden_state, feats, Wh, bh, Wf, bf, v):
    hidden_state = np.asarray(hidden_state, dtype=np.float32)
    feats = np.asarray(feats, dtype=np.float32)
    Wh = np.asarray(Wh, dtype=np.float32)
    bh = np.asarray(bh, dtype=np.float32)
    Wf = np.asarray(Wf, dtype=np.float32)
    bf = np.asarray(bf, dtype=np.float32)
    v = np.asarray(v, dtype=np.float32)

    WhT = np.ascontiguousarray(Wh.T)                      # [DH, DM]
    WfT = np.ascontiguousarray(Wf.T)                      # [DF, DM]
    WhT_bf = WhT.astype(ml_dtypes.bfloat16)
    WfT_bf = WfT.astype(ml_dtypes.bfloat16)
    bhfT = np.ascontiguousarray((bh + bf).reshape(MT, P).T)  # [128, 4]
    vT = np.ascontiguousarray(v.reshape(MT, P).T)         # [128, 4]

    in_maps = []
    for b in range(NCORES):
        in_maps.append({
            "hiddenT": np.ascontiguousarray(hidden_state[b].T).astype(ml_dtypes.bfloat16),
            "featsT": np.ascontiguousarray(feats[b].T).astype(ml_dtypes.bfloat16),
            "feats": np.ascontiguousarray(feats[b]).astype(ml_dtypes.bfloat16),
            "WhT": WhT_bf,
            "WfT": WfT_bf,
            "bhfT": bhfT,
            "vT": vT,
        })
    return in_maps


def kernel(hidden_state, feats, Wh, bh, Wf, bf, v, _run_kwargs=None):
    nc = _get_nc()
    in_maps = _prep_in_maps(hidden_state, feats, Wh, bh, Wf, bf, v)
    res = run_bass_kernel_spmd(nc, in_maps, list(range(NCORES)),
                               **(_run_kwargs or {}))
    context = np.stack([res.results[b]["context"] for b in range(NCORES)])
    weight = np.stack([res.results[b]["weight"] for b in range(NCORES)])
    kernel._last_results = res
    return context, weight
